# revision 45
# baseline (speedup 1.0000x reference)
"""MixHopVolatilityNet Trainium2 kernel (8 NeuronCores, SPMD).

Strategy (graph/data parallel, per sharding hint):
 - Nodes partitioned across 8 cores (1250 each) via a degree-balanced
   permutation; each core owns the destination side of every propagation
   for its nodes. Weights replicated.
 - Halo exchange: after each hop every core AllGathers its 1250-row slab
   into the next full [10000, F] feature table (on-chip ncfw collective).
 - Every hop runs as gather + segment matmul: a SWDGE dma_gather pulls the
   (deduplicated, per-128-dst-node-block) source rows of the replicated
   table into SBUF k-tiles (1024 rows / 8 k-tiles per instruction, the
   descriptor-ring limit), then PE contracts them against a host-built
   sparse weight block.
 - GCN weight factorization: w_e = dinv_src * dinv_dst. Tables store
   dinv_src-prescaled features and the PSUM->SBUF copies scale by dinv_dst
   (both folded into copies that exist anyway), so the segment-weight
   blocks hold small integer edge COUNTS - exactly representable in
   fp8e4m3. The fp8 hops then run DoubleRow fp8xfp8 matmuls (2 k-tiles
   per instruction at 0.5 cycles/row) with no accuracy loss from weights.
 - Layer 0 propagates h directly (propagate-then-project, 3x256-wide hops).
   Layers 1-2 project first (out_p = A^p (h @ W_p)), batching powers into
   [u1|u2|u3] so hops are 768/512/256 wide instead of 3x1024; the four
   power projections run as two 512-wide matmul chains per block.
 - The wide-hop tables (768/512) are staged as scaled fp8e4m3 - halves
   gather/AllGather volume at >=512B per gathered row (the DMA descriptor
   efficiency knee); 256-wide tables stay fp16 (fp8 would pay the sub-512B
   2x descriptor latency and add noise for zero DMA gain).
 - Schedule is a single software pipeline so the DMA engines never idle
   at phase boundaries: h0 is computed feature-major in wide matmul/gelu
   chunks (no per-block chains); each layer's LAST hop is fused per-block
   with layernorm, gelu, the NEXT layer's staged power projections and z1
   staging (pipelined two blocks deep so PE never waits on the gelu
   chain); the p=0 projection is deferred into the DMA-bound z1-hop
   phase; the final MLP runs per 512-column chunk inside layer 2's last
   hop. Projection weights drip into PE-bound phases in 256KB chunks (a
   monolithic 2MB load would stall the gather stream ~6us). A dummy
   matmul chain at t=0 warms the PE pstate while constants load.
 - Layernorm: two-pass bn_stats/bn_aggr in fp32; rsigma = (var+eps)^-0.5
   computed entirely on DVE (0x5f3759df bit-trick seed + two Newton
   iterations, ~4e-6 relative error) so the ACT activation-table stays
   pinned to Gelu - no per-block table reloads; normalize folded into the
   erf-gelu ACT op as gelu(x * rsigma - mu * rsigma).
 - AllGather stand-in HBM writes (timing build) are batched with the slab
   staging write via a broadcast-source 3D AP (one DMA per table per
   block) and spread across blocks so the halo table completes almost as
   soon as the last block stages.
"""

import heapq
import sys

import numpy as np

sys.path.insert(0, "/opt/trn_rl_repo")

# ---- problem constants (hardcoded per contract) ----
N = 10000
E = 160000
F_IN = 84
H = 256
P4 = 4
L = 3
PH = P4 * H  # 1024
NC = 8
NB = N // NC          # 1250 nodes per core
BLK = 128
NBLK = (NB + BLK - 1) // BLK   # 10 blocks; the last one holds 98 nodes
LAST = NB - (NBLK - 1) * BLK   # 98
EPS = 1e-5

# fp8 staging scales for the wide hop tables (z1: projections u1..u3,
# z2: A-propagated u2..u3). Values are O(1); scale into e4m3's sweet spot.
S_Z1 = 4.0
S_Z2 = 4.0
TABLE_F8 = True

# AllGather accounting for the cost-model estimate (width_elems, elem_bytes)
# in issue order: l0h0, l0h1, l0h2, then per layer 1,2: z1, z2, z3.
_zb1 = 1 if TABLE_F8 else 2
AG_SPECS = ([(H, 2)] * 3 + [(3 * H, _zb1), (2 * H, _zb1), (H, 2)] * 2)


def _nb_of(b):
    return min(BLK, NB - b * BLK)


# ----------------------------------------------------------------------------
# Host-side preprocessing
# ----------------------------------------------------------------------------

def _balance_nodes(wt):
    """Greedy LPT assignment of nodes to the 80 (core, block) bins so the
    per-block gather work is balanced. Returns perm: orig node -> new id."""
    nbins = NC * NBLK
    cap = np.full(nbins, BLK, np.int64)
    cap[NBLK - 1:: NBLK] = LAST
    order = np.argsort(-wt, kind="stable")
    heap = [(0, b) for b in range(nbins)]
    heapq.heapify(heap)
    fill = np.zeros(nbins, np.int64)
    perm = np.empty(N, np.int64)
    base = np.arange(nbins) // NBLK * NB + np.arange(nbins) % NBLK * BLK
    for node in order:
        while True:
            load, b = heapq.heappop(heap)
            if fill[b] < cap[b]:
                break
        perm[node] = base[b] + fill[b]
        fill[b] += 1
        if fill[b] < cap[b]:
            heapq.heappush(heap, (load + int(wt[node]), b))
    return perm


def _graph_prep(edge_index):
    """Build per-core gather index arrays and dense segment-weight blocks,
    with dst-side node balancing and per-block source deduplication."""
    src = edge_index[0].astype(np.int64)
    dst = edge_index[1].astype(np.int64)
    deg = np.bincount(dst, minlength=N).astype(np.float64) + 1.0
    dinv = deg ** -0.5
    loop = np.arange(N, dtype=np.int64)
    esrc = np.concatenate([src, loop])
    edst = np.concatenate([dst, loop])
    perm = _balance_nodes(deg)  # deg ~ per-dst gather row count
    psrc = perm[esrc]
    pdst = perm[edst]

    core = pdst // NB
    loc = pdst - core * NB
    blk = loc // BLK
    m = loc - blk * BLK
    gid = core * NBLK + blk
    order = np.argsort(gid, kind="stable")
    psrc, m, gid = psrc[order], m[order], gid[order]
    starts = np.searchsorted(gid, np.arange(NC * NBLK))
    ends = np.concatenate([starts[1:], [len(gid)]])

    # per-block dedup of gather sources
    uniq_lists = []
    kk = np.empty(len(gid), np.int64)
    counts = np.empty(NC * NBLK, np.int64)
    for g in range(NC * NBLK):
        s, e = starts[g], ends[g]
        u, inv = np.unique(psrc[s:e], return_inverse=True)
        uniq_lists.append(u)
        kk[s:e] = inv
        counts[g] = len(u)

    k_pad = int(np.ceil(max(counts.max(), 128) / 128.0) * 128)
    T = k_pad // 128

    # The GCN weight factors: w_e = dinv_src * dinv_dst. Tables store
    # dinv_src-prescaled features and psum outputs are scaled by dinv_dst,
    # so the segment-weight blocks hold small integer edge COUNTS — exactly
    # representable in fp8e4m3, enabling exact DoubleRow fp8 matmuls.
    wcnt = np.zeros((NC, 128, NBLK, T, BLK), np.float32)
    core_g = gid // NBLK
    blk_g = gid % NBLK
    np.add.at(wcnt, (core_g, kk % 128, blk_g, kk // 128, m), 1.0)
    assert wcnt.max() <= 15, "edge multiplicity too large for exact fp8"
    import ml_dtypes
    wcnt = wcnt.astype(ml_dtypes.float8_e4m3)

    # per-(core, block, slot) dinv of the permuted dst nodes
    dinv_p = np.ones(NC * NB, np.float32)
    dinv_p[perm] = dinv.astype(np.float32)
    # columns: 0 dinv, 1 dinv^2, 2 dinv*S_Z1, 3 dinv/S_Z1,
    #          4 dinv^2*S_Z2/S_Z1, 5 dinv/S_Z2, 6 dinv^2/S_Z2
    dv = np.ones((NC, 128, NBLK, 8), np.float32)
    for c in range(NC):
        for b in range(NBLK):
            nb = min(BLK, NB - b * BLK)
            r = dinv_p[c * NB + b * BLK: c * NB + b * BLK + nb]
            dv[c, :nb, b, 0] = r
            dv[c, :nb, b, 1] = r * r
            dv[c, :nb, b, 2] = r * S_Z1
            dv[c, :nb, b, 3] = r / S_Z1
            dv[c, :nb, b, 4] = r * r * S_Z2 / S_Z1
            dv[c, :nb, b, 5] = r / S_Z2
            dv[c, :nb, b, 6] = r * r / S_Z2

    idxs = np.zeros((NC, NBLK, k_pad), np.int64)
    for g in range(NC * NBLK):
        u = uniq_lists[g]
        idxs[g // NBLK, g % NBLK, : len(u)] = u
    tbmax = tuple(int(x) for x in counts.reshape(NC, NBLK).max(axis=0))

    # dma_gather layout: chunks of <=1024 idxs (8 k-tiles), each wrapped
    # in 16 partitions and replicated across the 8 GPSIMD cores:
    # idx16[c, p, b, ch, j] = idxs[c, b, ch*1024 + j*16 + p%16]
    NCH = (T + 7) // 8
    kp2 = NCH * 1024
    if kp2 > k_pad:
        idxs = np.concatenate(
            [idxs, np.zeros((NC, NBLK, kp2 - k_pad), np.int64)], axis=2)
    wrapped = idxs.reshape(NC, NBLK, NCH, 64, 16)       # [c,b,ch,j,p16]
    wrapped = wrapped.transpose(0, 4, 1, 2, 3)          # [c,p16,b,ch,j]
    idx16 = np.tile(wrapped, (1, 8, 1, 1, 1)).astype(np.int16)
    return wcnt, dv, idx16, k_pad, tbmax, perm


def _w_moving(w):
    """[K, Nout] -> moving layout [128, Kt, Nout] fp16 (partition = K % 128)."""
    K, Nout = w.shape
    Kt = (K + 127) // 128
    out = np.zeros((128, Kt, Nout), np.float16)
    for t in range(Kt):
        rows = w[t * 128: min((t + 1) * 128, K)]
        out[: rows.shape[0], t] = rows.astype(np.float16)
    return out


def _w_stationary(w):
    """[K, M] -> stationary tiles [128, Kt, Mt, 128] fp16."""
    K, M = w.shape
    Kt = (K + 127) // 128
    Mt = (M + 127) // 128
    out = np.zeros((128, Kt, Mt, 128), np.float16)
    for t in range(Kt):
        for u in range(Mt):
            blk = w[t * 128: min((t + 1) * 128, K),
                    u * 128: min((u + 1) * 128, M)].astype(np.float16)
            out[: blk.shape[0], t, u, : blk.shape[1]] = blk
    return out


# ----------------------------------------------------------------------------
# Bass program
# ----------------------------------------------------------------------------

def _build_nc(k_pad, tbmax, nontriv, use_collectives=True):
    import concourse.bacc as bacc
    import concourse.bass as bass  # noqa: F401
    import concourse.mybir as mybir
    import concourse.tile as tile
    from concourse.alu_op_type import AluOpType
    from contextlib import ExitStack

    f16 = mybir.dt.float16
    f32 = mybir.dt.float32
    f8 = mybir.dt.float8e4
    i16 = mybir.dt.int16
    AF = mybir.ActivationFunctionType
    T = k_pad // 128
    NCH = (T + 7) // 8
    RG = [list(range(NC))]

    nc = bacc.Bacc("TRN2", target_bir_lowering=False, debug=False,
                   num_devices=NC)

    # ---- I/O ----
    xT_d = nc.dram_tensor("xT", [F_IN, NB], f16, kind="ExternalInput")
    idx_d = nc.dram_tensor("idx16", [128, NBLK, NCH, 64], i16,
                           kind="ExternalInput")
    wseg_d = nc.dram_tensor("wsegT", [128, NBLK, T, BLK], f8,
                            kind="ExternalInput")
    dinv_d = nc.dram_tensor("dinv_c", [128, NBLK, 8], f32,
                            kind="ExternalInput")
    w_in_d = nc.dram_tensor("w_in_st", [128, 1, 2, 128], f16,
                            kind="ExternalInput")
    w0_d = nc.dram_tensor("w0_m", [P4, 128, 2, H], f16, kind="ExternalInput")
    w12_d = nc.dram_tensor("w12_m", [2, P4, 128, 8, H], f16,
                           kind="ExternalInput")
    w1_d = nc.dram_tensor("w1_st", [128, 8, 2, 128], f16, kind="ExternalInput")
    w2_d = nc.dram_tensor("w2_st", [128, 2, 1, 128], f16, kind="ExternalInput")
    w3_d = nc.dram_tensor("w3_st", [128, 1], f16, kind="ExternalInput")
    ident_d = nc.dram_tensor("ident", [128, 128], f16, kind="ExternalInput")
    if nontriv["b_in"]:
        b_in_d = nc.dram_tensor("b_in_fm", [128, 2], f32,
                                kind="ExternalInput")
        b_in_bc_d = nc.dram_tensor("b_in_bc", [128, H], f32,
                                   kind="ExternalInput")
    if nontriv["bcat"]:
        bcat_d = nc.dram_tensor("bcat_bc", [L, 128, PH], f32,
                                kind="ExternalInput")
    if nontriv["ln"]:
        lng_d = nc.dram_tensor("lng_bc", [L, 128, PH], f32,
                               kind="ExternalInput")
        lnb_d = nc.dram_tensor("lnb_bc", [L, 128, PH], f32,
                               kind="ExternalInput")
    if nontriv["b1"]:
        b1_d = nc.dram_tensor("b1_c", [128, 2], f32, kind="ExternalInput")
    if nontriv["b2"]:
        b2_d = nc.dram_tensor("b2_c", [128, 1], f32, kind="ExternalInput")
    y_d = nc.dram_tensor("y_out", [NB], f32, kind="ExternalOutput")

    # ---- internal DRAM: AG inputs (local) and gather tables (shared) ----
    # (name, width, dtype, table scale): wide z tables are scaled fp8.
    zdt = f8 if TABLE_F8 else f16
    tspec = {"l0h0": (H, f16, 1.0), "l0h1": (H, f16, 1.0),
             "l0h2": (H, f16, 1.0)}
    for lyr in (1, 2):
        tspec[f"l{lyr}z1"] = (3 * H, zdt, S_Z1 if TABLE_F8 else 1.0)
        tspec[f"l{lyr}z2"] = (2 * H, zdt, S_Z2 if TABLE_F8 else 1.0)
        tspec[f"l{lyr}z3"] = (H, f16, 1.0)
    ag_in = {}
    table = {}
    for name, (width, dt, _s) in tspec.items():
        ag_in[name] = nc.dram_tensor(f"agin_{name}", [NB, width], dt)
        table[name] = nc.dram_tensor(f"tab_{name}", [N, width], dt,
                                     addr_space="Shared")

    with tile.TileContext(nc) as tc, ExitStack() as ctx:
        const = ctx.enter_context(tc.tile_pool(name="const", bufs=1))
        work = ctx.enter_context(tc.tile_pool(name="work", bufs=8))
        gpool = ctx.enter_context(tc.tile_pool(name="gpool", bufs=3))
        scp = ctx.enter_context(tc.tile_pool(name="scp", bufs=10))
        zpool = ctx.enter_context(tc.tile_pool(name="zpool", bufs=4))
        big = ctx.enter_context(tc.tile_pool(name="big", bufs=1))
        gath = ctx.enter_context(tc.tile_pool(name="gath", bufs=4))
        one = ctx.enter_context(tc.tile_pool(name="one", bufs=1))
        psum = ctx.enter_context(tc.tile_pool(name="psum", bufs=6,
                                              space="PSUM"))
        pstr = ctx.enter_context(tc.tile_pool(name="pstr", bufs=2,
                                              space="PSUM"))

        # ---- early SBUF constants (everything stage0/l0-hop needs) ----
        xT_sb = const.tile([F_IN, NB], f16, tag="xT")
        nc.sync.dma_start(out=xT_sb[:], in_=xT_d[:])
        w_in_sb = const.tile([128, 1, 2, 128], f16, tag="w_in")
        nc.sync.dma_start(out=w_in_sb[:], in_=w_in_d[:])
        ident_sb = const.tile([128, 128], f16, tag="ident")
        nc.sync.dma_start(out=ident_sb[:], in_=ident_d[:])
        zero_sb = const.tile([128, 1], f32, tag="zero")
        nc.vector.memset(zero_sb[:], 0.0)
        idx_sb = const.tile([128, NBLK, NCH, 64], i16, tag="idx")
        nc.sync.dma_start(out=idx_sb[:], in_=idx_d[:])
        dinv_sb = const.tile([128, NBLK, 8], f32, tag="dinv")
        nc.sync.dma_start(out=dinv_sb[:], in_=dinv_d[:])
        wseg_sb = const.tile([128, NBLK, T, BLK], f8, tag="wseg")
        nc.sync.dma_start(
            out=wseg_sb[:, 0:2].rearrange("p a t b -> p (a t b)"),
            in_=wseg_d[:, 0:2].rearrange("p a t b -> p (a t b)"))
        w0_sb = const.tile([128, P4, 2, H], f16, tag="w0")
        if nontriv["b_in"]:
            b_in_sb = const.tile([128, 2], f32, tag="b_in")
            nc.sync.dma_start(out=b_in_sb[:], in_=b_in_d[:])

        # persistent activations. During layer 0, hT[:, 2p:2p+2, :] holds the
        # feature-major transpose of A^p h (the hops' projection operands);
        # after each layernorm it holds the feature-major layer output.
        hT = big.tile([128, 8, NB], f16, tag="hT")
        hcat = big.tile([128, NBLK, PH], f16, tag="hcat")

        # PE pstate warmup: ~4us of dummy matmuls while the constants load
        # (PE would idle anyway). The cost model runs matmuls at 2-4x cycle
        # time until the engine has been continuously busy for 3us; this
        # keeps stage0's matmuls and transposes at full clock.
        nc.vector.memset(hT[:, 0, 0:640], 0.0)
        wps = psum.tile([128, 512], f32, tag="mm", name="wps")
        for i in range(8):
            nc.tensor.matmul(wps[:, :512],
                             hT[:, 0, 512:640],
                             hT[:, 0, 0:512],
                             start=(i == 0), stop=(i == 7))

        def zb(nb):
            return zero_sb[:nb, 0:1]

        pending_agins = []

        def stage_ag(name, b, src_ap, nb, defer=False):
            """Write block b's slab rows into ag_in[name]; in the timing
            build, also write the AllGather's stand-in HBM volume (2x slab,
            same total bytes as the established estimate) as ONE
            broadcast-source DMA so the halo table completes almost as soon
            as the last block is staged. The table write goes first on the
            sync queue (it gates the next hop's gathers); the ag_in write
            rides the scalar queue. With defer=True the ag_in write is
            queued until flush_agins() (just before the AllGather) so the
            gating table writes get the HWDGE to themselves."""
            if not use_collectives:
                tab3 = table[name].rearrange("(c n) w -> n c w", c=NC)
                nc.sync.dma_start(
                    out=tab3[b * BLK: b * BLK + nb, 0:2, :],
                    in_=src_ap.unsqueeze(1).broadcast_to(
                        [nb, 2, src_ap.shape[-1]]))
            if defer:
                pending_agins.append((name, b, src_ap, nb))
            else:
                nc.scalar.dma_start(
                    out=ag_in[name][b * BLK: b * BLK + nb, :], in_=src_ap)

        def flush_agins():
            for (name, b, src_ap, nb) in pending_agins:
                nc.scalar.dma_start(
                    out=ag_in[name][b * BLK: b * BLK + nb, :], in_=src_ap)
            pending_agins.clear()

        def allgather(name):
            """Halo exchange ag_in[name] -> table[name] (on-chip ncfw
            collective; the cost-model build accounts it via stage_ag +
            the analytic estimate)."""
            if use_collectives:
                nc.gpsimd.collective_compute(
                    "AllGather", AluOpType.bypass, replica_groups=RG,
                    ins=[ag_in[name][:]], outs=[table[name][:]],
                )

        def transpose_pack(slot0, srcs, b, nb):
            """Transpose each src [nb, 128] into one PSUM bank tile, then
            strided DVE copies move them into hT[:, slot0:...] feature-major
            (split in two so downstream consumers of the first k-tiles start
            half a copy earlier). Packing amortizes the copy overhead and
            decouples PE from DVE pacing."""
            nkt = len(srcs)
            pst = pstr.tile([128, 8, 128], f16, tag="tr")
            for kt, src_ap in enumerate(srcs):
                nc.tensor.transpose(pst[:, kt, :nb], src_ap,
                                    ident_sb[:nb, :nb])
            h1 = (nkt + 1) // 2
            nc.vector.tensor_copy(
                hT[:, slot0:slot0 + h1, b * BLK: b * BLK + nb],
                pst[:, :h1, :nb])
            if h1 < nkt:
                nc.vector.tensor_copy(
                    hT[:, slot0 + h1:slot0 + nkt, b * BLK: b * BLK + nb],
                    pst[:, h1:nkt, :nb])

        def seg_psums(name, b):
            """Propagation block b: dma_gather the (deduplicated) source rows
            of table[name] in 8-ktile chunks, contract against wsegT on PE.
            Returns [(c0, cw, psum_tile)]."""
            width, dt, _s = tspec[name]
            tab = table[name]
            outs = []
            c0 = 0
            while c0 < width:
                cw = min(512, width - c0)
                ps = psum.tile([128, 512], f32, tag="mm", name="ps_seg")
                outs.append((c0, cw, ps))
                c0 += cw
            wmax = max(w for (w, d, _s) in tspec.values() if d == dt)
            cnt = tbmax[b]
            Tb = (cnt + 127) // 128
            # 16-granular trim skips pad-row transfers. The first hop per
            # table dtype stays 128-granular (full k-tile writes) so every
            # gather buffer byte is initialized before trimmed gathers can
            # leave (zero-count) stale tails.
            first = name in ("l0h0", "l1z1")
            gran = 128 if first else 16
            for ch in range(NCH):
                nidx = min(1024, max(0, ((cnt + gran - 1) // gran * gran)
                                     - ch * 1024))
                if nidx == 0:
                    break
                nk = (nidx + 127) // 128
                kt0 = ch * 8
                gt = gath.tile([128, 8 * wmax], dt, tag=f"gt_{dt}",
                               name="gt")
                nc.gpsimd.dma_gather(
                    out_ap=gt[:, : nk * width].rearrange(
                        "p (a w) -> p a w", w=width),
                    in_ap=tab[:],
                    idxs_ap=idx_sb[:, b, ch, : nidx // 16],
                    num_idxs=nidx, num_idxs_reg=nidx,
                    elem_size=width)
                gt3 = gt[:, : nk * width].rearrange("p (a w) -> p a w",
                                                    w=width)
                for (c0, cw, ps) in outs:
                    kt = kt0
                    while kt < kt0 + nk:
                        if dt == f8 and kt + 1 < kt0 + nk:
                            nc.tensor.matmul(
                                ps[:, :cw],
                                wseg_sb[:, b, kt: kt + 2, :],
                                gt3[:, kt - kt0: kt - kt0 + 2, c0: c0 + cw],
                                start=(kt == 0),
                                stop=(kt + 1 == Tb - 1),
                                perf_mode=mybir.MatmulPerfMode.DoubleRow,
                            )
                            kt += 2
                        else:
                            o = (kt - kt0) * width + c0
                            nc.tensor.matmul(
                                ps[:, :cw],
                                wseg_sb[:, b, kt, :],
                                gt[:, o: o + cw],
                                start=(kt == 0),
                                stop=(kt == Tb - 1),
                            )
                            kt += 1
            return outs

        mvs = {}
        sts = {}

        def ln_stats_half0(layer, b):
            """First bn_stats half: hcat[:, b, 0:512] is complete well before
            the block's last-hop output lands, so this runs off the critical
            chain."""
            hc = hcat[:, b, :]
            if nontriv["bcat"]:
                nc.vector.tensor_tensor(hc, hc, bcat_sb[:, layer, :],
                                        AluOpType.add)
            st = work.tile([128, 12], f32, tag=f"bnst{b % 3}", name="st")
            nc.vector.bn_stats(st[:, 0:6], hcat[:, b, 0:512])
            sts[b] = st

        def ln_stats(layer, b):
            """Layernorm pass 1 tail: second bn_stats half, aggregate,
            rsigma and -mu*rsigma. rsigma = (var+eps)^-0.5 runs entirely on
            DVE — bit-trick seed (0x5f3759df) + two Newton iterations,
            ~4e-6 relative error — so the ACT activation table stays pinned
            to Gelu (no per-block table reloads)."""
            i32 = mybir.dt.int32
            st = sts.pop(b)
            nc.vector.bn_stats(st[:, 6:12], hcat[:, b, 512:1024])
            mv = work.tile([128, 8], f32, tag=f"bnmv{b}", name="mv")
            nc.vector.bn_aggr(mv[:, 0:2], st[:])
            # v = var + eps; y0 bits = 0x5f3759df - (v_bits >> 1)
            nc.vector.tensor_scalar_add(mv[:, 2:3], mv[:, 1:2], EPS)
            nc.vector.tensor_scalar(mv[:, 6:7].bitcast(i32),
                                    mv[:, 2:3].bitcast(i32), 1, None,
                                    AluOpType.arith_shift_right)
            nc.vector.tensor_scalar(mv[:, 3:4].bitcast(i32),
                                    mv[:, 6:7].bitcast(i32), -1, 0x5f3759df,
                                    AluOpType.mult, AluOpType.add)
            for _ in range(2):
                nc.vector.tensor_tensor(mv[:, 6:7], mv[:, 3:4], mv[:, 3:4],
                                        AluOpType.mult)
                nc.vector.tensor_tensor(mv[:, 6:7], mv[:, 6:7], mv[:, 2:3],
                                        AluOpType.mult)
                nc.vector.tensor_scalar(mv[:, 6:7], mv[:, 6:7], -0.5, 1.5,
                                        AluOpType.mult, AluOpType.add)
                nc.vector.tensor_tensor(mv[:, 3:4], mv[:, 3:4], mv[:, 6:7],
                                        AluOpType.mult)
            nc.vector.tensor_tensor(mv[:, 4:5], mv[:, 0:1], mv[:, 3:4],
                                    AluOpType.mult)
            nc.vector.tensor_scalar_mul(mv[:, 5:6], mv[:, 4:5], -1.0)
            mvs[b] = mv

        gls = {}

        def ln_gelu(layer, b):
            """Per-block layernorm pass 2 fused into the erf-gelu ACT op
            when there is no affine term: gelu(x * rsigma + (-mu * rsigma)),
            split in halves so downstream transposes can start early."""
            mv = mvs[b]
            gl = gpool.tile([128, PH], f16, tag="gel")
            if nontriv["ln"]:
                xn = one.tile([128, PH], f32, tag="xn")
                nc.vector.tensor_scalar(
                    xn[:], hcat[:, b, :], mv[:, 0:1], mv[:, 3:4],
                    AluOpType.subtract, AluOpType.mult,
                )
                nc.vector.tensor_tensor(xn[:], xn[:],
                                        lng_sb[:, layer, :],
                                        AluOpType.mult)
                nc.vector.tensor_tensor(xn[:], xn[:],
                                        lnb_sb[:, layer, :],
                                        AluOpType.add)
                nc.scalar.activation(gl[:, 0:512], xn[:, 0:512], AF.Gelu,
                                     bias=zb(128))
                nc.scalar.activation(gl[:, 512:1024], xn[:, 512:1024],
                                     AF.Gelu, bias=zb(128))
            else:
                nc.scalar.activation(gl[:, 0:512], hcat[:, b, 0:512],
                                     AF.Gelu, bias=mv[:, 5:6],
                                     scale=mv[:, 3:4])
                nc.scalar.activation(gl[:, 512:1024], hcat[:, b, 512:1024],
                                     AF.Gelu, bias=mv[:, 5:6],
                                     scale=mv[:, 3:4])
            gls[b] = gl

        def ln_transposes(b):
            """Move gelu(ln(x)) for block b into feature-major hT."""
            nb = _nb_of(b)
            gl = gls.pop(b)
            transpose_pack(0, [gl[:nb, kt * 128:(kt + 1) * 128]
                               for kt in range(8)], b, nb)

        def l0_project_block(p, b):
            """hcat[:, b, p*H:(p+1)*H] = h_p @ mh_w0[p] from hT[:, 2p:2p+2]."""
            nb = _nb_of(b)
            ps = psum.tile([128, 512], f32, tag="mm")
            for kt in range(2):
                nc.tensor.matmul(ps[:nb, :H],
                                 hT[:, 2 * p + kt, b * BLK: b * BLK + nb],
                                 w0_sb[:, p, kt, :],
                                 start=(kt == 0), stop=(kt == 1))
            nc.vector.tensor_copy(hcat[:nb, b, p * H:(p + 1) * H],
                                  ps[:nb, :H])

        # ================= stage 0: h0 = gelu(x @ w_in + b_in) =============
        # Two independent paths off one xT operand: a node-major per-block
        # matmul+gelu chain feeds the dinv-prescaled staging rows (shortest
        # possible path to the first AllGather — no transposes), while
        # feature-major matmul/gelu chunks fill hT[:, 0:2, :] for the layer-0
        # projections off the critical path.
        chunks = [(c, min(512, NB - c)) for c in range(0, NB, 512)]
        for (c0, cw) in chunks:
            for mt in range(2):
                ps = psum.tile([128, 512], f32, tag="mm")
                nc.tensor.matmul(ps[:, :cw], w_in_sb[:F_IN, 0, mt, :],
                                 xT_sb[:, c0:c0 + cw], start=True, stop=True)
                bias = b_in_sb[:, mt:mt + 1] if nontriv["b_in"] else zb(128)
                nc.scalar.activation(hT[:, mt, c0:c0 + cw], ps[:, :cw],
                                     AF.Gelu, bias=bias)
        for b in range(NBLK):
            nb = _nb_of(b)
            sc = scp.tile([128, H], f16, tag="scst", name="sc")
            pst = pstr.tile([128, 8, 128], f16, tag="tr")
            for mt in range(2):
                nc.tensor.transpose(pst[:nb, mt, :],
                                    hT[:, mt, b * BLK: b * BLK + nb],
                                    ident_sb[:, :])
            nc.vector.tensor_scalar_mul(
                sc[:nb, :], pst[:nb, 0:2, :].rearrange("p a w -> p (a w)"),
                dinv_sb[:nb, b, 0:1])
            stage_ag("l0h0", b, sc[:nb, :], nb, defer=True)
        flush_agins()
        allgather("l0h0")

        # late constants: loaded in small chunks dripped one per block into
        # DMA-slack phases via `fillers` (a monolithic 2MB dma_start would
        # hold the DMA engines for ~6us and stall the gather stream).
        w12_sbs = [const.tile([128, P4, 8, H], f16, tag=f"w12_{li}",
                              name=f"w12s_{li}")
                   for li in range(2)]
        w1_sb = const.tile([128, 8, 2, 128], f16, tag="w1")
        w2_sb = const.tile([128, 2, 1, 128], f16, tag="w2")
        w3_sb = const.tile([128, 1], f16, tag="w3")
        # wseg tail + w0 ride the scalar queue during stage0 (its DMA stream
        # has slack); the big projection weights drip into the PE-bound
        # transition phases via `fillers`.
        for b0 in (2, 4, 6, 8):
            nc.scalar.dma_start(
                out=wseg_sb[:, b0:b0 + 2].rearrange("p a t b -> p (a t b)"),
                in_=wseg_d[:, b0:b0 + 2].rearrange("p a t b -> p (a t b)"))
        nc.scalar.dma_start(
            out=w0_sb[:].rearrange("q p t h -> q p (t h)"),
            in_=w0_d[:].rearrange("p q t h -> q p (t h)"))
        fillers = []

        def _w12_chunk(li, p, half):
            s = slice(half * 1024, (half + 1) * 1024)
            nc.scalar.dma_start(
                out=w12_sbs[li][:, p].rearrange("q t h -> q (t h)")[:, s],
                in_=w12_d[li, p].rearrange("q t h -> q (t h)")[:, s])

        for li in range(2):
            for p in range(P4):
                for half in range(2):
                    fillers.append(
                        lambda li=li, p=p, half=half: _w12_chunk(li, p, half))
        fillers.append(lambda: nc.scalar.dma_start(
            out=w1_sb[:, 0:4].rearrange("p t m w -> p t (m w)"),
            in_=w1_d[:, 0:4].rearrange("p t m w -> p t (m w)")))
        fillers.append(lambda: nc.scalar.dma_start(
            out=w1_sb[:, 4:8].rearrange("p t m w -> p t (m w)"),
            in_=w1_d[:, 4:8].rearrange("p t m w -> p t (m w)")))
        fillers.append(lambda: nc.scalar.dma_start(out=w2_sb[:],
                                                   in_=w2_d[:]))
        fillers.append(lambda: nc.scalar.dma_start(out=w3_sb[:],
                                                   in_=w3_d[:]))
        if nontriv["bcat"]:
            bcat_sb = const.tile([128, L, PH], f32, tag="bcat")
            fillers.append(lambda: nc.scalar.dma_start(
                out=bcat_sb[:], in_=bcat_d[:].rearrange("l p f -> p l f")))
        if nontriv["ln"]:
            lng_sb = const.tile([128, L, PH], f32, tag="lng")
            lnb_sb = const.tile([128, L, PH], f32, tag="lnb")
            fillers.append(lambda: nc.scalar.dma_start(
                out=lng_sb[:], in_=lng_d[:].rearrange("l p f -> p l f")))
            fillers.append(lambda: nc.scalar.dma_start(
                out=lnb_sb[:], in_=lnb_d[:].rearrange("l p f -> p l f")))
        if nontriv["b1"]:
            b1_sb = const.tile([128, 2], f32, tag="b1")
            fillers.append(lambda: nc.scalar.dma_start(out=b1_sb[:],
                                                       in_=b1_d[:]))
        if nontriv["b2"]:
            b2_sb = const.tile([128, 1], f32, tag="b2")
            fillers.append(lambda: nc.scalar.dma_start(out=b2_sb[:],
                                                       in_=b2_d[:]))

        def drain_filler(k=1):
            for _ in range(k):
                if fillers:
                    fillers.pop(0)()

        # ================= layer 0: propagate-then-project =================
        for b in range(NBLK):
            l0_project_block(0, b)
        for p, (tin, tout) in enumerate(
                [("l0h0", "l0h1"), ("l0h1", "l0h2")], start=1):
            for b in range(NBLK):
                nb = _nb_of(b)
                (_, _, ps), = seg_psums(tin, b)
                stg = work.tile([128, H], f16, tag="stage")
                nc.vector.tensor_scalar_mul(stg[:, :H], ps[:, :H],
                                            dinv_sb[:, b, 0:1])
                sc = scp.tile([128, H], f16, tag="scst", name="sc")
                nc.vector.tensor_scalar_mul(sc[:nb, :], ps[:nb, :H],
                                            dinv_sb[:nb, b, 1:2])
                stage_ag(tout, b, sc[:nb, :], nb, defer=True)
                transpose_pack(2 * p, [stg[:nb, kt * 128:(kt + 1) * 128]
                                       for kt in range(2)], b, nb)
                l0_project_block(p, b)
            flush_agins()
            allgather(tout)

        def proj_block(li, b):
            """Layer li+1 staged power projections (p=1..3) for block b from
            hT, scaled into the z1 staging row, then stage z1. The p=0
            projection is deferred into the (DMA-bound) z1-hop phase — its
            output is only read by the next layernorm. Powers are packed two
            per PSUM bank so each block uses 2 "mm" ring slots, leaving WAR
            slack for the next block's segment matmuls."""
            nb = _nb_of(b)
            zname = f"l{li + 1}z1"
            zdt1 = tspec[zname][1]
            w12_sb = w12_sbs[li]
            ztile = zpool.tile([128, PH], zdt1, tag="zstage")
            for pair in range(2):
                ps = psum.tile([128, 512], f32, tag="mm")
                for half in range(2):
                    p = 2 * pair + half
                    if p == 0:
                        continue
                    for kt in range(8):
                        nc.tensor.matmul(
                            ps[:nb, half * H:(half + 1) * H],
                            hT[:, kt, b * BLK: b * BLK + nb],
                            w12_sb[:, p, kt, :],
                            start=(kt == 0), stop=(kt == 7))
                if pair == 0:
                    nc.vector.tensor_scalar_mul(
                        ztile[:nb, 0:H], ps[:nb, H: 2 * H],
                        dinv_sb[:nb, b, 2:3])
                else:
                    nc.vector.tensor_scalar_mul(
                        ztile[:nb, H: 3 * H], ps[:nb, : 2 * H],
                        dinv_sb[:nb, b, 2:3])
            stage_ag(zname, b, ztile[:nb, : 3 * H], nb)

        def proj_p0(li, b):
            """Deferred p=0 projection into hcat[:, b, 0:H]."""
            nb = _nb_of(b)
            w12_sb = w12_sbs[li]
            ps = psum.tile([128, 512], f32, tag="mm")
            for kt in range(8):
                nc.tensor.matmul(ps[:nb, :H],
                                 hT[:, kt, b * BLK: b * BLK + nb],
                                 w12_sb[:, 0, kt, :],
                                 start=(kt == 0), stop=(kt == 7))
            nc.vector.tensor_copy(hcat[:nb, b, 0:H], ps[:nb, :H])

        # ======= fused transition: layer-i last hop -> layer-(i+1) projs ====
        # Per block: segment hop, hop output into hcat, ln stats + gelu
        # (DVE/ACT); one block later (so PE never waits on the gelu chain):
        # transposes into hT and the next layer's projections + z1 staging.
        def transition(layer):
            """layer 0: hop over l0h2 (power 3) + project into l1z1.
            layer 1: hop over l1z3 + project into l2z1."""
            def stage_a(b):
                nb = _nb_of(b)
                ln_stats_half0(layer, b)
                if layer == 0:
                    (_, _, ps), = seg_psums("l0h2", b)
                    drain_filler(4 if b < 2 else 2)
                    stg = work.tile([128, H], f16, tag="stage")
                    nc.vector.tensor_scalar_mul(stg[:, :H], ps[:, :H],
                                                dinv_sb[:, b, 0:1])
                    transpose_pack(6, [stg[:nb, kt * 128:(kt + 1) * 128]
                                       for kt in range(2)], b, nb)
                    l0_project_block(3, b)
                else:
                    (_, _, ps), = seg_psums("l1z3", b)
                    drain_filler(4 if b < 2 else 2)
                    nc.vector.tensor_scalar_mul(
                        hcat[:nb, b, 3 * H: 4 * H], ps[:nb, :H],
                        dinv_sb[:nb, b, 0:1])
                ln_stats(layer, b)
                ln_gelu(layer, b)

            def stage_b(b):
                ln_transposes(b)
                proj_block(layer, b)

            for b in range(NBLK):
                stage_a(b)
                if b >= 2:
                    stage_b(b - 2)
            stage_b(NBLK - 2)
            stage_b(NBLK - 1)
            allgather(f"l{layer + 1}z1")

        transition(0)

        # ================= layers 1-2: wide hops ===========================
        def wide_hops(layer):
            """Hops 768 -> 512 over the scaled fp8 z tables. PSUM carries
            s_in * A z_in; copies out rescale: hcat gets 1/s_in, staging
            gets s_out/s_in."""
            zname = [f"l{layer}z1", f"l{layer}z2", f"l{layer}z3"]
            for hop in range(2):
                width = (3 - hop) * H
                tin, tout = zname[hop], zname[hop + 1]
                for b in range(NBLK):
                    nb = _nb_of(b)
                    pieces = seg_psums(tin, b)
                    if hop == 0:
                        proj_p0(layer - 1, b)
                    hc_col = {0: 3, 1: 5}[hop]
                    nc.vector.tensor_scalar_mul(
                        hcat[:nb, b, (hop + 1) * H:(hop + 2) * H],
                        pieces[0][2][:nb, :H],
                        dinv_sb[:nb, b, hc_col: hc_col + 1])
                    zdt_o = tspec[tout][1]
                    stg = zpool.tile([128, 2 * H], zdt_o, tag="zhstage")
                    st_col = 4 if hop == 0 else 6
                    for (c0, cw, ps) in pieces:
                        if c0 + cw <= H:
                            continue
                        lo = max(H, c0)
                        nc.vector.tensor_scalar_mul(
                            stg[:nb, lo - H: c0 + cw - H],
                            ps[:nb, lo - c0: cw],
                            dinv_sb[:nb, b, st_col: st_col + 1])
                    stage_ag(tout, b, stg[:nb, : width - H], nb)
                allgather(tout)

        wide_hops(1)
        transition(1)
        wide_hops(2)

        # ==== final: layer-2 last hop fused with layernorm + MLP chunks ====
        m1T = big.tile([128, 2, NB], f16, tag="m1T")
        m2T = big.tile([128, NB], f16, tag="m2T")
        ysb = big.tile([1, NB], f32, tag="ysb")

        def mlp_chunk(c0, cw):
            for mt in range(2):
                ps = psum.tile([128, 512], f32, tag="mm")
                for kt in range(8):
                    nc.tensor.matmul(ps[:, :cw], w1_sb[:, kt, mt, :],
                                     hT[:, kt, c0:c0 + cw],
                                     start=(kt == 0), stop=(kt == 7))
                bias = b1_sb[:, mt:mt + 1] if nontriv["b1"] else zb(128)
                nc.scalar.activation(m1T[:, mt, c0:c0 + cw], ps[:, :cw],
                                     AF.Gelu, bias=bias)
            ps = psum.tile([128, 512], f32, tag="mm")
            for kt in range(2):
                nc.tensor.matmul(ps[:, :cw], w2_sb[:, kt, 0, :],
                                 m1T[:, kt, c0:c0 + cw],
                                 start=(kt == 0), stop=(kt == 1))
            bias = b2_sb[:, 0:1] if nontriv["b2"] else zb(128)
            nc.scalar.activation(m2T[:, c0:c0 + cw], ps[:, :cw],
                                 AF.Gelu, bias=bias)
            ps = psum.tile([128, 512], f32, tag="mm")
            nc.tensor.matmul(ps[:1, :cw], w3_sb[:, :1], m2T[:, c0:c0 + cw],
                             start=True, stop=True)
            nc.vector.tensor_copy(ysb[:1, c0:c0 + cw], ps[:1, :cw])

        def final_a(b):
            nb = _nb_of(b)
            ln_stats_half0(2, b)
            (_, _, ps), = seg_psums("l2z3", b)
            drain_filler(2)
            nc.vector.tensor_scalar_mul(
                hcat[:nb, b, 3 * H: 4 * H], ps[:nb, :H],
                dinv_sb[:nb, b, 0:1])
            ln_stats(2, b)
            ln_gelu(2, b)

        done_cols = [0]

        def final_b(b):
            ln_transposes(b)
            ready = min((b + 1) * BLK, NB)
            while done_cols[0] + 512 <= ready or (b == NBLK - 1
                                                 and done_cols[0] < NB):
                c0 = done_cols[0]
                cw = min(512, NB - c0)
                mlp_chunk(c0, cw)
                done_cols[0] = c0 + cw

        for b in range(NBLK):
            final_a(b)
            if b >= 1:
                final_b(b - 1)
        final_b(NBLK - 1)
        nc.sync.dma_start(out=y_d[:], in_=ysb[:1, :])

    nc.compile()
    return nc


# ----------------------------------------------------------------------------
# Public entry point
# ----------------------------------------------------------------------------

_CACHE = {}


def _prep_inputs(inputs):
    x = np.asarray(inputs["x"], np.float32)
    edge_index = np.asarray(inputs["edge_index"])
    wcnt, dvec, idx16, k_pad, tbmax, perm = _graph_prep(edge_index)

    b3 = np.asarray(inputs["b3"], np.float32)
    nontriv = {
        "b_in": bool(np.any(inputs["b_in"])),
        "bcat": bool(np.any(inputs["mh_b0"]) or np.any(inputs["mh_b12"])),
        "ln": not (np.allclose(np.asarray(inputs["ln_g"]), 1.0)
                   and not np.any(inputs["ln_b"])),
        "b1": bool(np.any(inputs["b1"])),
        "b2": bool(np.any(inputs["b2"])),
    }

    shared = {
        "w_in_st": _w_stationary(np.asarray(inputs["w_in"], np.float32)),
        "w0_m": np.stack([_w_moving(np.asarray(inputs["mh_w0"][p], np.float32))
                          for p in range(P4)]),
        "w12_m": np.stack([
            np.stack([_w_moving(np.asarray(inputs["mh_w12"][l, p], np.float32))
                      for p in range(P4)])
            for l in range(2)]),
        "w1_st": _w_stationary(np.asarray(inputs["w1"], np.float32)),
        "w2_st": _w_stationary(np.asarray(inputs["w2"], np.float32)),
        "w3_st": np.asarray(inputs["w3"], np.float32).astype(np.float16),
        "ident": np.eye(128, dtype=np.float16),
    }
    if nontriv["b_in"]:
        shared["b_in_fm"] = np.ascontiguousarray(
            np.asarray(inputs["b_in"], np.float32).reshape(2, 128).T)
        shared["b_in_bc"] = np.tile(np.asarray(inputs["b_in"], np.float32),
                                    (128, 1))
    if nontriv["bcat"]:
        bcat = np.zeros((L, PH), np.float32)
        bcat[0] = np.asarray(inputs["mh_b0"], np.float32).reshape(-1)
        bcat[1] = np.asarray(inputs["mh_b12"], np.float32)[0].reshape(-1)
        bcat[2] = np.asarray(inputs["mh_b12"], np.float32)[1].reshape(-1)
        shared["bcat_bc"] = np.ascontiguousarray(
            np.broadcast_to(bcat[:, None, :], (L, 128, PH)))
    if nontriv["ln"]:
        shared["lng_bc"] = np.ascontiguousarray(np.broadcast_to(
            np.asarray(inputs["ln_g"], np.float32)[:, None, :], (L, 128, PH)))
        shared["lnb_bc"] = np.ascontiguousarray(np.broadcast_to(
            np.asarray(inputs["ln_b"], np.float32)[:, None, :], (L, 128, PH)))
    if nontriv["b1"]:
        shared["b1_c"] = np.ascontiguousarray(
            np.asarray(inputs["b1"], np.float32).reshape(2, 128).T)
    if nontriv["b2"]:
        shared["b2_c"] = np.asarray(inputs["b2"], np.float32).reshape(128, 1)

    xp = x[np.argsort(perm)]  # xp[newid] = x[orig]
    in_maps = []
    for c in range(NC):
        m = dict(shared)
        m["xT"] = np.ascontiguousarray(
            xp[c * NB:(c + 1) * NB].T.astype(np.float16))
        m["idx16"] = np.ascontiguousarray(idx16[c])
        m["wsegT"] = np.ascontiguousarray(wcnt[c])
        m["dinv_c"] = np.ascontiguousarray(dvec[c])
        in_maps.append(m)
    return in_maps, k_pad, tbmax, nontriv, b3, perm


def _run(inputs, trace=False, **kwargs):
    from concourse.bass_utils import run_bass_kernel_spmd

    in_maps, k_pad, tbmax, nontriv, b3, perm = _prep_inputs(inputs)
    key = (k_pad, tbmax, tuple(sorted(nontriv.items())))
    if key not in _CACHE:
        _CACHE[key] = _build_nc(k_pad, tbmax, nontriv)
    nc = _CACHE[key]
    res = run_bass_kernel_spmd(nc, in_maps, list(range(NC)), trace=trace,
                               **kwargs)
    ycat = np.concatenate([res.results[c]["y_out"] for c in range(NC)])
    y = ycat[perm].astype(np.float32) + b3.reshape(-1)[0]
    return y, res


def kernel(**inputs) -> np.ndarray:
    y, _ = _run(inputs, trace=False)
    return y


# revision 49
# speedup vs baseline: 1.0134x; 1.0134x over previous
"""MixHopVolatilityNet Trainium2 kernel (8 NeuronCores, SPMD).

Strategy (graph/data parallel, per sharding hint):
 - Nodes partitioned across 8 cores (1250 each) via a degree-balanced
   permutation; each core owns the destination side of every propagation
   for its nodes. Weights replicated.
 - Halo exchange: after each hop every core AllGathers its 1250-row slab
   into the next full [10000, F] feature table (on-chip ncfw collective).
 - Every hop runs as gather + segment matmul: a SWDGE dma_gather pulls the
   (deduplicated, per-128-dst-node-block) source rows of the replicated
   table into SBUF k-tiles (1024 rows / 8 k-tiles per instruction, the
   descriptor-ring limit), then PE contracts them against a host-built
   sparse weight block.
 - GCN weight factorization: w_e = dinv_src * dinv_dst. Tables store
   dinv_src-prescaled features and the PSUM->SBUF copies scale by dinv_dst
   (both folded into copies that exist anyway), so the segment-weight
   blocks hold small integer edge COUNTS - exactly representable in
   fp8e4m3. The fp8 hops then run DoubleRow fp8xfp8 matmuls (2 k-tiles
   per instruction at 0.5 cycles/row) with no accuracy loss from weights.
 - Layer 0 propagates h directly (propagate-then-project, 3x256-wide hops).
   Layers 1-2 project first (out_p = A^p (h @ W_p)), batching powers into
   [u1|u2|u3] so hops are 768/512/256 wide instead of 3x1024; the four
   power projections run as two 512-wide matmul chains per block.
 - The wide-hop tables (768/512) are staged as scaled fp8e4m3 - halves
   gather/AllGather volume at >=512B per gathered row (the DMA descriptor
   efficiency knee); 256-wide tables stay fp16 (fp8 would pay the sub-512B
   2x descriptor latency and add noise for zero DMA gain).
 - Schedule is a single software pipeline so the DMA engines never idle
   at phase boundaries: h0 is computed feature-major in wide matmul/gelu
   chunks (no per-block chains); each layer's LAST hop is fused per-block
   with layernorm, gelu, the NEXT layer's staged power projections and z1
   staging (pipelined two blocks deep so PE never waits on the gelu
   chain); the p=0 projection is deferred into the DMA-bound z1-hop
   phase; the final MLP runs per 512-column chunk inside layer 2's last
   hop. Projection weights drip into PE-bound phases in 256KB chunks (a
   monolithic 2MB load would stall the gather stream ~6us). A dummy
   matmul chain at t=0 warms the PE pstate while constants load.
 - Layernorm: two-pass bn_stats/bn_aggr in fp32; rsigma = (var+eps)^-0.5
   computed entirely on DVE (0x5f3759df bit-trick seed + two Newton
   iterations, ~4e-6 relative error) so the ACT activation-table stays
   pinned to Gelu - no per-block table reloads; normalize folded into the
   erf-gelu ACT op as gelu(x * rsigma - mu * rsigma).
 - AllGather stand-in HBM writes (timing build) are batched with the slab
   staging write via a broadcast-source 3D AP (one DMA per table per
   block) and spread across blocks so the halo table completes almost as
   soon as the last block stages.
"""

import heapq
import sys

import numpy as np

sys.path.insert(0, "/opt/trn_rl_repo")

# ---- problem constants (hardcoded per contract) ----
N = 10000
E = 160000
F_IN = 84
H = 256
P4 = 4
L = 3
PH = P4 * H  # 1024
NC = 8
NB = N // NC          # 1250 nodes per core
BLK = 128
NBLK = (NB + BLK - 1) // BLK   # 10 blocks; the last one holds 98 nodes
LAST = NB - (NBLK - 1) * BLK   # 98
EPS = 1e-5

# fp8 staging scales for the wide hop tables (z1: projections u1..u3,
# z2: A-propagated u2..u3). Values are O(1); scale into e4m3's sweet spot.
S_Z1 = 4.0
S_Z2 = 4.0
TABLE_F8 = True

# AllGather accounting for the cost-model estimate (width_elems, elem_bytes)
# in issue order: l0h0, l0h1, l0h2, then per layer 1,2: z1, z2, z3.
_zb1 = 1 if TABLE_F8 else 2
AG_SPECS = ([(H, 2)] * 3 + [(3 * H, _zb1), (2 * H, _zb1), (H, 2)] * 2)


def _nb_of(b):
    return min(BLK, NB - b * BLK)


# ----------------------------------------------------------------------------
# Host-side preprocessing
# ----------------------------------------------------------------------------

def _balance_nodes(wt):
    """Greedy LPT assignment of nodes to the 80 (core, block) bins so the
    per-block gather work is balanced. Returns perm: orig node -> new id."""
    nbins = NC * NBLK
    cap = np.full(nbins, BLK, np.int64)
    cap[NBLK - 1:: NBLK] = LAST
    order = np.argsort(-wt, kind="stable")
    heap = [(0, b) for b in range(nbins)]
    heapq.heapify(heap)
    fill = np.zeros(nbins, np.int64)
    perm = np.empty(N, np.int64)
    base = np.arange(nbins) // NBLK * NB + np.arange(nbins) % NBLK * BLK
    for node in order:
        while True:
            load, b = heapq.heappop(heap)
            if fill[b] < cap[b]:
                break
        perm[node] = base[b] + fill[b]
        fill[b] += 1
        if fill[b] < cap[b]:
            heapq.heappush(heap, (load + int(wt[node]), b))
    return perm


def _graph_prep(edge_index):
    """Build per-core gather index arrays and dense segment-weight blocks,
    with dst-side node balancing and per-block source deduplication."""
    src = edge_index[0].astype(np.int64)
    dst = edge_index[1].astype(np.int64)
    deg = np.bincount(dst, minlength=N).astype(np.float64) + 1.0
    dinv = deg ** -0.5
    loop = np.arange(N, dtype=np.int64)
    esrc = np.concatenate([src, loop])
    edst = np.concatenate([dst, loop])
    perm = _balance_nodes(deg)  # deg ~ per-dst gather row count
    psrc = perm[esrc]
    pdst = perm[edst]

    core = pdst // NB
    loc = pdst - core * NB
    blk = loc // BLK
    m = loc - blk * BLK
    gid = core * NBLK + blk
    order = np.argsort(gid, kind="stable")
    psrc, m, gid = psrc[order], m[order], gid[order]
    starts = np.searchsorted(gid, np.arange(NC * NBLK))
    ends = np.concatenate([starts[1:], [len(gid)]])

    # per-block dedup of gather sources
    uniq_lists = []
    kk = np.empty(len(gid), np.int64)
    counts = np.empty(NC * NBLK, np.int64)
    for g in range(NC * NBLK):
        s, e = starts[g], ends[g]
        u, inv = np.unique(psrc[s:e], return_inverse=True)
        uniq_lists.append(u)
        kk[s:e] = inv
        counts[g] = len(u)

    k_pad = int(np.ceil(max(counts.max(), 128) / 128.0) * 128)
    T = k_pad // 128

    # The GCN weight factors: w_e = dinv_src * dinv_dst. Tables store
    # dinv_src-prescaled features and psum outputs are scaled by dinv_dst,
    # so the segment-weight blocks hold small integer edge COUNTS — exactly
    # representable in fp8e4m3, enabling exact DoubleRow fp8 matmuls.
    wcnt = np.zeros((NC, 128, NBLK, T, BLK), np.float32)
    core_g = gid // NBLK
    blk_g = gid % NBLK
    np.add.at(wcnt, (core_g, kk % 128, blk_g, kk // 128, m), 1.0)
    assert wcnt.max() <= 15, "edge multiplicity too large for exact fp8"
    import ml_dtypes
    wcnt = wcnt.astype(ml_dtypes.float8_e4m3)

    # per-(core, block, slot) dinv of the permuted dst nodes
    dinv_p = np.ones(NC * NB, np.float32)
    dinv_p[perm] = dinv.astype(np.float32)
    # columns: 0 dinv, 1 dinv^2, 2 dinv*S_Z1, 3 dinv/S_Z1,
    #          4 dinv^2*S_Z2/S_Z1, 5 dinv/S_Z2, 6 dinv^2/S_Z2
    dv = np.ones((NC, 128, NBLK, 8), np.float32)
    for c in range(NC):
        for b in range(NBLK):
            nb = min(BLK, NB - b * BLK)
            r = dinv_p[c * NB + b * BLK: c * NB + b * BLK + nb]
            dv[c, :nb, b, 0] = r
            dv[c, :nb, b, 1] = r * r
            dv[c, :nb, b, 2] = r * S_Z1
            dv[c, :nb, b, 3] = r / S_Z1
            dv[c, :nb, b, 4] = r * r * S_Z2 / S_Z1
            dv[c, :nb, b, 5] = r / S_Z2
            dv[c, :nb, b, 6] = r * r / S_Z2

    idxs = np.zeros((NC, NBLK, k_pad), np.int64)
    for g in range(NC * NBLK):
        u = uniq_lists[g]
        idxs[g // NBLK, g % NBLK, : len(u)] = u
    tbmax = tuple(int(x) for x in counts.reshape(NC, NBLK).max(axis=0))

    # dma_gather layout: chunks of <=1024 idxs (8 k-tiles), each wrapped
    # in 16 partitions and replicated across the 8 GPSIMD cores:
    # idx16[c, p, b, ch, j] = idxs[c, b, ch*1024 + j*16 + p%16]
    NCH = (T + 7) // 8
    kp2 = NCH * 1024
    if kp2 > k_pad:
        idxs = np.concatenate(
            [idxs, np.zeros((NC, NBLK, kp2 - k_pad), np.int64)], axis=2)
    wrapped = idxs.reshape(NC, NBLK, NCH, 64, 16)       # [c,b,ch,j,p16]
    wrapped = wrapped.transpose(0, 4, 1, 2, 3)          # [c,p16,b,ch,j]
    idx16 = np.tile(wrapped, (1, 8, 1, 1, 1)).astype(np.int16)
    return wcnt, dv, idx16, k_pad, tbmax, perm


def _w_moving(w):
    """[K, Nout] -> moving layout [128, Kt, Nout] fp16 (partition = K % 128)."""
    K, Nout = w.shape
    Kt = (K + 127) // 128
    out = np.zeros((128, Kt, Nout), np.float16)
    for t in range(Kt):
        rows = w[t * 128: min((t + 1) * 128, K)]
        out[: rows.shape[0], t] = rows.astype(np.float16)
    return out


def _w_stationary(w):
    """[K, M] -> stationary tiles [128, Kt, Mt, 128] fp16."""
    K, M = w.shape
    Kt = (K + 127) // 128
    Mt = (M + 127) // 128
    out = np.zeros((128, Kt, Mt, 128), np.float16)
    for t in range(Kt):
        for u in range(Mt):
            blk = w[t * 128: min((t + 1) * 128, K),
                    u * 128: min((u + 1) * 128, M)].astype(np.float16)
            out[: blk.shape[0], t, u, : blk.shape[1]] = blk
    return out


# ----------------------------------------------------------------------------
# Bass program
# ----------------------------------------------------------------------------

def _build_nc(k_pad, tbmax, nontriv, use_collectives=True):
    import concourse.bacc as bacc
    import concourse.bass as bass  # noqa: F401
    import concourse.mybir as mybir
    import concourse.tile as tile
    from concourse.alu_op_type import AluOpType
    from contextlib import ExitStack

    f16 = mybir.dt.float16
    f32 = mybir.dt.float32
    f8 = mybir.dt.float8e4
    i16 = mybir.dt.int16
    AF = mybir.ActivationFunctionType
    T = k_pad // 128
    NCH = (T + 7) // 8
    RG = [list(range(NC))]

    nc = bacc.Bacc("TRN2", target_bir_lowering=False, debug=False,
                   num_devices=NC)

    # ---- I/O ----
    xT_d = nc.dram_tensor("xT", [F_IN, NB], f16, kind="ExternalInput")
    idx_d = nc.dram_tensor("idx16", [128, NBLK, NCH, 64], i16,
                           kind="ExternalInput")
    wseg_d = nc.dram_tensor("wsegT", [128, NBLK, T, BLK], f8,
                            kind="ExternalInput")
    dinv_d = nc.dram_tensor("dinv_c", [128, NBLK, 8], f32,
                            kind="ExternalInput")
    w_in_d = nc.dram_tensor("w_in_st", [128, 1, 2, 128], f16,
                            kind="ExternalInput")
    w0_d = nc.dram_tensor("w0_m", [P4, 128, 2, H], f16, kind="ExternalInput")
    w12_d = nc.dram_tensor("w12_m", [2, P4, 128, 8, H], f16,
                           kind="ExternalInput")
    w1_d = nc.dram_tensor("w1_st", [128, 8, 2, 128], f16, kind="ExternalInput")
    w2_d = nc.dram_tensor("w2_st", [128, 2, 1, 128], f16, kind="ExternalInput")
    w3_d = nc.dram_tensor("w3_st", [128, 1], f16, kind="ExternalInput")
    ident_d = nc.dram_tensor("ident", [128, 128], f16, kind="ExternalInput")
    if nontriv["b_in"]:
        b_in_d = nc.dram_tensor("b_in_fm", [128, 2], f32,
                                kind="ExternalInput")
        b_in_bc_d = nc.dram_tensor("b_in_bc", [128, H], f32,
                                   kind="ExternalInput")
    if nontriv["bcat"]:
        bcat_d = nc.dram_tensor("bcat_bc", [L, 128, PH], f32,
                                kind="ExternalInput")
    if nontriv["ln"]:
        lng_d = nc.dram_tensor("lng_bc", [L, 128, PH], f32,
                               kind="ExternalInput")
        lnb_d = nc.dram_tensor("lnb_bc", [L, 128, PH], f32,
                               kind="ExternalInput")
    if nontriv["b1"]:
        b1_d = nc.dram_tensor("b1_c", [128, 2], f32, kind="ExternalInput")
    if nontriv["b2"]:
        b2_d = nc.dram_tensor("b2_c", [128, 1], f32, kind="ExternalInput")
    y_d = nc.dram_tensor("y_out", [NB], f32, kind="ExternalOutput")

    # ---- internal DRAM: AG inputs (local) and gather tables (shared) ----
    # (name, width, dtype, table scale): wide z tables are scaled fp8.
    zdt = f8 if TABLE_F8 else f16
    tspec = {"l0h0": (H, f16, 1.0), "l0h1": (H, f16, 1.0),
             "l0h2": (H, f16, 1.0)}
    for lyr in (1, 2):
        tspec[f"l{lyr}z1"] = (3 * H, zdt, S_Z1 if TABLE_F8 else 1.0)
        tspec[f"l{lyr}z2"] = (2 * H, zdt, S_Z2 if TABLE_F8 else 1.0)
        tspec[f"l{lyr}z3"] = (H, f16, 1.0)
    ag_in = {}
    table = {}
    for name, (width, dt, _s) in tspec.items():
        ag_in[name] = nc.dram_tensor(f"agin_{name}", [NBLK * BLK, width],
                                     dt)
        table[name] = nc.dram_tensor(f"tab_{name}", [N, width], dt,
                                     addr_space="Shared")

    with tile.TileContext(nc) as tc, ExitStack() as ctx:
        const = ctx.enter_context(tc.tile_pool(name="const", bufs=1))
        work = ctx.enter_context(tc.tile_pool(name="work", bufs=8))
        gpool = ctx.enter_context(tc.tile_pool(name="gpool", bufs=3))
        zpool = ctx.enter_context(tc.tile_pool(name="zpool", bufs=4))
        big = ctx.enter_context(tc.tile_pool(name="big", bufs=1))
        gath = ctx.enter_context(tc.tile_pool(name="gath", bufs=4))
        one = ctx.enter_context(tc.tile_pool(name="one", bufs=1))
        psum = ctx.enter_context(tc.tile_pool(name="psum", bufs=6,
                                              space="PSUM"))
        pstr = ctx.enter_context(tc.tile_pool(name="pstr", bufs=2,
                                              space="PSUM"))

        # ---- early SBUF constants (everything stage0/l0-hop needs) ----
        xT_sb = const.tile([F_IN, NB], f16, tag="xT")
        nc.sync.dma_start(out=xT_sb[:], in_=xT_d[:])
        w_in_sb = const.tile([128, 1, 2, 128], f16, tag="w_in")
        nc.sync.dma_start(out=w_in_sb[:], in_=w_in_d[:])
        ident_sb = const.tile([128, 128], f16, tag="ident")
        nc.sync.dma_start(out=ident_sb[:], in_=ident_d[:])
        zero_sb = const.tile([128, 1], f32, tag="zero")
        nc.vector.memset(zero_sb[:], 0.0)
        idx_sb = const.tile([128, NBLK, NCH, 64], i16, tag="idx")
        nc.sync.dma_start(out=idx_sb[:], in_=idx_d[:])
        dinv_sb = const.tile([128, NBLK, 8], f32, tag="dinv")
        nc.sync.dma_start(out=dinv_sb[:], in_=dinv_d[:])
        wseg_sb = const.tile([128, NBLK, T, BLK], f8, tag="wseg")
        nc.sync.dma_start(
            out=wseg_sb[:, 0:2].rearrange("p a t b -> p (a t b)"),
            in_=wseg_d[:, 0:2].rearrange("p a t b -> p (a t b)"))
        w0_sb = const.tile([128, P4, 2, H], f16, tag="w0")
        if nontriv["b_in"]:
            b_in_sb = const.tile([128, 2], f32, tag="b_in")
            nc.sync.dma_start(out=b_in_sb[:], in_=b_in_d[:])

        # persistent activations. During layer 0, hT[:, 2p:2p+2, :] holds the
        # feature-major transpose of A^p h (the hops' projection operands);
        # after each layernorm it holds the feature-major layer output.
        hT = big.tile([128, 8, NB], f16, tag="hT")
        hcat = big.tile([128, NBLK, PH], f16, tag="hcat")
        scall = big.tile([128, NBLK, H], f16, tag="scall")
        nc.vector.memset(scall[:, NBLK - 1, :], 0.0)

        # PE pstate warmup: ~4us of dummy matmuls while the constants load
        # (PE would idle anyway). The cost model runs matmuls at 2-4x cycle
        # time until the engine has been continuously busy for 3us; this
        # keeps stage0's matmuls and transposes at full clock.
        nc.vector.memset(hT[:, 0, 0:640], 0.0)
        wps = psum.tile([128, 512], f32, tag="mm", name="wps")
        for i in range(8):
            nc.tensor.matmul(wps[:, :512],
                             hT[:, 0, 512:640],
                             hT[:, 0, 0:512],
                             start=(i == 0), stop=(i == 7))

        def zb(nb):
            return zero_sb[:nb, 0:1]

        def stage_ag(name, b, src_ap, nb, agin=True):
            """Write block b's slab rows into ag_in[name]; in the timing
            build, also write the AllGather's stand-in HBM volume (2x slab,
            same total bytes as the established estimate) as ONE
            broadcast-source DMA so the halo table completes almost as soon
            as the last block is staged. The table write goes first on the
            sync queue (it gates the next hop's gathers); the ag_in write
            rides the scalar queue. Tables staged via the persistent scall
            buffer skip the per-block ag_in write (one batched DMA at
            flush_agin_big instead, keeping HWDGE clear for the gating
            table writes)."""
            if not use_collectives:
                tab3 = table[name].rearrange("(c n) w -> n c w", c=NC)
                nc.sync.dma_start(
                    out=tab3[b * BLK: b * BLK + nb, 0:2, :],
                    in_=src_ap.unsqueeze(1).broadcast_to(
                        [nb, 2, src_ap.shape[-1]]))
            if agin:
                nc.scalar.dma_start(
                    out=ag_in[name][b * BLK: b * BLK + nb, :], in_=src_ap)

        def flush_agin_big(name):
            """One batched ag_in write for a table staged via scall."""
            nc.scalar.dma_start(
                out=ag_in[name][:].rearrange("(blk p) w -> p blk w", p=BLK),
                in_=scall[:, :, :])

        def allgather(name):
            """Halo exchange ag_in[name] -> table[name] (on-chip ncfw
            collective; the cost-model build accounts it via stage_ag +
            the analytic estimate)."""
            if use_collectives:
                nc.gpsimd.collective_compute(
                    "AllGather", AluOpType.bypass, replica_groups=RG,
                    ins=[ag_in[name][0:NB, :]], outs=[table[name][:]],
                )

        def transpose_pack(slot0, srcs, b, nb):
            """Transpose each src [nb, 128] into one PSUM bank tile, then
            strided DVE copies move them into hT[:, slot0:...] feature-major
            (split in two so downstream consumers of the first k-tiles start
            half a copy earlier). Packing amortizes the copy overhead and
            decouples PE from DVE pacing."""
            nkt = len(srcs)
            pst = pstr.tile([128, 8, 128], f16, tag="tr")
            for kt, src_ap in enumerate(srcs):
                nc.tensor.transpose(pst[:, kt, :nb], src_ap,
                                    ident_sb[:nb, :nb])
            h1 = (nkt + 1) // 2
            nc.vector.tensor_copy(
                hT[:, slot0:slot0 + h1, b * BLK: b * BLK + nb],
                pst[:, :h1, :nb])
            if h1 < nkt:
                nc.vector.tensor_copy(
                    hT[:, slot0 + h1:slot0 + nkt, b * BLK: b * BLK + nb],
                    pst[:, h1:nkt, :nb])

        def seg_psums(name, b):
            """Propagation block b: dma_gather the (deduplicated) source rows
            of table[name] in 8-ktile chunks, contract against wsegT on PE.
            Returns [(c0, cw, psum_tile)]."""
            width, dt, _s = tspec[name]
            tab = table[name]
            outs = []
            c0 = 0
            while c0 < width:
                cw = min(512, width - c0)
                ps = psum.tile([128, 512], f32, tag="mm", name="ps_seg")
                outs.append((c0, cw, ps))
                c0 += cw
            wmax = max(w for (w, d, _s) in tspec.values() if d == dt)
            cnt = tbmax[b]
            Tb = (cnt + 127) // 128
            # 16-granular trim skips pad-row transfers. The first hop per
            # table dtype stays 128-granular (full k-tile writes) so every
            # gather buffer byte is initialized before trimmed gathers can
            # leave (zero-count) stale tails.
            first = name in ("l0h0", "l1z1")
            gran = 128 if first else 16
            for ch in range(NCH):
                nidx = min(1024, max(0, ((cnt + gran - 1) // gran * gran)
                                     - ch * 1024))
                if nidx == 0:
                    break
                nk = (nidx + 127) // 128
                kt0 = ch * 8
                gt = gath.tile([128, 8 * wmax], dt, tag=f"gt_{dt}",
                               name="gt")
                nc.gpsimd.dma_gather(
                    out_ap=gt[:, : nk * width].rearrange(
                        "p (a w) -> p a w", w=width),
                    in_ap=tab[:],
                    idxs_ap=idx_sb[:, b, ch, : nidx // 16],
                    num_idxs=nidx, num_idxs_reg=nidx,
                    elem_size=width)
                gt3 = gt[:, : nk * width].rearrange("p (a w) -> p a w",
                                                    w=width)
                for (c0, cw, ps) in outs:
                    kt = kt0
                    while kt < kt0 + nk:
                        if dt == f8 and kt + 1 < kt0 + nk:
                            nc.tensor.matmul(
                                ps[:, :cw],
                                wseg_sb[:, b, kt: kt + 2, :],
                                gt3[:, kt - kt0: kt - kt0 + 2, c0: c0 + cw],
                                start=(kt == 0),
                                stop=(kt + 1 == Tb - 1),
                                perf_mode=mybir.MatmulPerfMode.DoubleRow,
                            )
                            kt += 2
                        else:
                            o = (kt - kt0) * width + c0
                            nc.tensor.matmul(
                                ps[:, :cw],
                                wseg_sb[:, b, kt, :],
                                gt[:, o: o + cw],
                                start=(kt == 0),
                                stop=(kt == Tb - 1),
                            )
                            kt += 1
            return outs

        mvs = {}
        sts = {}

        def ln_stats_half0(layer, b):
            """First bn_stats half: hcat[:, b, 0:512] is complete well before
            the block's last-hop output lands, so this runs off the critical
            chain."""
            hc = hcat[:, b, :]
            if nontriv["bcat"]:
                nc.vector.tensor_tensor(hc, hc, bcat_sb[:, layer, :],
                                        AluOpType.add)
            st = work.tile([128, 12], f32, tag=f"bnst{b % 3}", name="st")
            nc.vector.bn_stats(st[:, 0:6], hcat[:, b, 0:512])
            sts[b] = st

        def ln_stats(layer, b):
            """Layernorm pass 1 tail: second bn_stats half, aggregate,
            rsigma and -mu*rsigma. rsigma = (var+eps)^-0.5 runs entirely on
            DVE — bit-trick seed (0x5f3759df) + two Newton iterations,
            ~4e-6 relative error — so the ACT activation table stays pinned
            to Gelu (no per-block table reloads)."""
            i32 = mybir.dt.int32
            st = sts.pop(b)
            nc.vector.bn_stats(st[:, 6:12], hcat[:, b, 512:1024])
            mv = work.tile([128, 8], f32, tag=f"bnmv{b}", name="mv")
            nc.vector.bn_aggr(mv[:, 0:2], st[:])
            # v = var + eps; y0 bits = 0x5f3759df - (v_bits >> 1)
            nc.vector.tensor_scalar_add(mv[:, 2:3], mv[:, 1:2], EPS)
            nc.vector.tensor_scalar(mv[:, 6:7].bitcast(i32),
                                    mv[:, 2:3].bitcast(i32), 1, None,
                                    AluOpType.arith_shift_right)
            nc.vector.tensor_scalar(mv[:, 3:4].bitcast(i32),
                                    mv[:, 6:7].bitcast(i32), -1, 0x5f3759df,
                                    AluOpType.mult, AluOpType.add)
            for _ in range(2):
                nc.vector.tensor_tensor(mv[:, 6:7], mv[:, 3:4], mv[:, 3:4],
                                        AluOpType.mult)
                nc.vector.tensor_tensor(mv[:, 6:7], mv[:, 6:7], mv[:, 2:3],
                                        AluOpType.mult)
                nc.vector.tensor_scalar(mv[:, 6:7], mv[:, 6:7], -0.5, 1.5,
                                        AluOpType.mult, AluOpType.add)
                nc.vector.tensor_tensor(mv[:, 3:4], mv[:, 3:4], mv[:, 6:7],
                                        AluOpType.mult)
            nc.vector.tensor_tensor(mv[:, 4:5], mv[:, 0:1], mv[:, 3:4],
                                    AluOpType.mult)
            nc.vector.tensor_scalar_mul(mv[:, 5:6], mv[:, 4:5], -1.0)
            mvs[b] = mv

        gls = {}

        def ln_gelu(layer, b):
            """Per-block layernorm pass 2 fused into the erf-gelu ACT op
            when there is no affine term: gelu(x * rsigma + (-mu * rsigma)),
            split in halves so downstream transposes can start early."""
            mv = mvs[b]
            gl = gpool.tile([128, PH], f16, tag="gel")
            if nontriv["ln"]:
                xn = one.tile([128, PH], f32, tag="xn")
                nc.vector.tensor_scalar(
                    xn[:], hcat[:, b, :], mv[:, 0:1], mv[:, 3:4],
                    AluOpType.subtract, AluOpType.mult,
                )
                nc.vector.tensor_tensor(xn[:], xn[:],
                                        lng_sb[:, layer, :],
                                        AluOpType.mult)
                nc.vector.tensor_tensor(xn[:], xn[:],
                                        lnb_sb[:, layer, :],
                                        AluOpType.add)
                nc.scalar.activation(gl[:, 0:512], xn[:, 0:512], AF.Gelu,
                                     bias=zb(128))
                nc.scalar.activation(gl[:, 512:1024], xn[:, 512:1024],
                                     AF.Gelu, bias=zb(128))
            else:
                nc.scalar.activation(gl[:, 0:512], hcat[:, b, 0:512],
                                     AF.Gelu, bias=mv[:, 5:6],
                                     scale=mv[:, 3:4])
                nc.scalar.activation(gl[:, 512:1024], hcat[:, b, 512:1024],
                                     AF.Gelu, bias=mv[:, 5:6],
                                     scale=mv[:, 3:4])
            gls[b] = gl

        def ln_transposes(b):
            """Move gelu(ln(x)) for block b into feature-major hT."""
            nb = _nb_of(b)
            gl = gls.pop(b)
            transpose_pack(0, [gl[:nb, kt * 128:(kt + 1) * 128]
                               for kt in range(8)], b, nb)

        def l0_project_block(p, b):
            """hcat[:, b, p*H:(p+1)*H] = h_p @ mh_w0[p] from hT[:, 2p:2p+2]."""
            nb = _nb_of(b)
            ps = psum.tile([128, 512], f32, tag="mm")
            for kt in range(2):
                nc.tensor.matmul(ps[:nb, :H],
                                 hT[:, 2 * p + kt, b * BLK: b * BLK + nb],
                                 w0_sb[:, p, kt, :],
                                 start=(kt == 0), stop=(kt == 1))
            nc.vector.tensor_copy(hcat[:nb, b, p * H:(p + 1) * H],
                                  ps[:nb, :H])

        # ================= stage 0: h0 = gelu(x @ w_in + b_in) =============
        # Two independent paths off one xT operand: a node-major per-block
        # matmul+gelu chain feeds the dinv-prescaled staging rows (shortest
        # possible path to the first AllGather — no transposes), while
        # feature-major matmul/gelu chunks fill hT[:, 0:2, :] for the layer-0
        # projections off the critical path.
        chunks = [(c, min(512, NB - c)) for c in range(0, NB, 512)]
        for (c0, cw) in chunks:
            for mt in range(2):
                ps = psum.tile([128, 512], f32, tag="mm")
                nc.tensor.matmul(ps[:, :cw], w_in_sb[:F_IN, 0, mt, :],
                                 xT_sb[:, c0:c0 + cw], start=True, stop=True)
                bias = b_in_sb[:, mt:mt + 1] if nontriv["b_in"] else zb(128)
                nc.scalar.activation(hT[:, mt, c0:c0 + cw], ps[:, :cw],
                                     AF.Gelu, bias=bias)
        for b in range(NBLK):
            nb = _nb_of(b)
            pst = pstr.tile([128, 8, 128], f16, tag="tr")
            for mt in range(2):
                nc.tensor.transpose(pst[:nb, mt, :],
                                    hT[:, mt, b * BLK: b * BLK + nb],
                                    ident_sb[:, :])
            nc.vector.tensor_scalar_mul(
                scall[:nb, b, :],
                pst[:nb, 0:2, :].rearrange("p a w -> p (a w)"),
                dinv_sb[:nb, b, 0:1])
            stage_ag("l0h0", b, scall[:nb, b, :], nb, agin=False)
        flush_agin_big("l0h0")
        allgather("l0h0")

        # late constants: loaded in small chunks dripped one per block into
        # DMA-slack phases via `fillers` (a monolithic 2MB dma_start would
        # hold the DMA engines for ~6us and stall the gather stream).
        w12_sbs = [const.tile([128, P4, 8, H], f16, tag=f"w12_{li}",
                              name=f"w12s_{li}")
                   for li in range(2)]
        w1_sb = const.tile([128, 8, 2, 128], f16, tag="w1")
        w2_sb = const.tile([128, 2, 1, 128], f16, tag="w2")
        w3_sb = const.tile([128, 1], f16, tag="w3")
        # wseg tail + w0 ride the scalar queue during stage0 (its DMA stream
        # has slack); the big projection weights drip into the PE-bound
        # transition phases via `fillers`.
        for b0 in (2, 4, 6, 8):
            nc.scalar.dma_start(
                out=wseg_sb[:, b0:b0 + 2].rearrange("p a t b -> p (a t b)"),
                in_=wseg_d[:, b0:b0 + 2].rearrange("p a t b -> p (a t b)"))
        nc.scalar.dma_start(
            out=w0_sb[:].rearrange("q p t h -> q p (t h)"),
            in_=w0_d[:].rearrange("p q t h -> q p (t h)"))
        fillers = []

        def _w12_chunk(li, p, half):
            s = slice(half * 1024, (half + 1) * 1024)
            nc.scalar.dma_start(
                out=w12_sbs[li][:, p].rearrange("q t h -> q (t h)")[:, s],
                in_=w12_d[li, p].rearrange("q t h -> q (t h)")[:, s])

        for li in range(2):
            for p in range(P4):
                for half in range(2):
                    fillers.append(
                        lambda li=li, p=p, half=half: _w12_chunk(li, p, half))
        fillers.append(lambda: nc.scalar.dma_start(
            out=w1_sb[:, 0:4].rearrange("p t m w -> p t (m w)"),
            in_=w1_d[:, 0:4].rearrange("p t m w -> p t (m w)")))
        fillers.append(lambda: nc.scalar.dma_start(
            out=w1_sb[:, 4:8].rearrange("p t m w -> p t (m w)"),
            in_=w1_d[:, 4:8].rearrange("p t m w -> p t (m w)")))
        fillers.append(lambda: nc.scalar.dma_start(out=w2_sb[:],
                                                   in_=w2_d[:]))
        fillers.append(lambda: nc.scalar.dma_start(out=w3_sb[:],
                                                   in_=w3_d[:]))
        if nontriv["bcat"]:
            bcat_sb = const.tile([128, L, PH], f32, tag="bcat")
            fillers.append(lambda: nc.scalar.dma_start(
                out=bcat_sb[:], in_=bcat_d[:].rearrange("l p f -> p l f")))
        if nontriv["ln"]:
            lng_sb = const.tile([128, L, PH], f32, tag="lng")
            lnb_sb = const.tile([128, L, PH], f32, tag="lnb")
            fillers.append(lambda: nc.scalar.dma_start(
                out=lng_sb[:], in_=lng_d[:].rearrange("l p f -> p l f")))
            fillers.append(lambda: nc.scalar.dma_start(
                out=lnb_sb[:], in_=lnb_d[:].rearrange("l p f -> p l f")))
        if nontriv["b1"]:
            b1_sb = const.tile([128, 2], f32, tag="b1")
            fillers.append(lambda: nc.scalar.dma_start(out=b1_sb[:],
                                                       in_=b1_d[:]))
        if nontriv["b2"]:
            b2_sb = const.tile([128, 1], f32, tag="b2")
            fillers.append(lambda: nc.scalar.dma_start(out=b2_sb[:],
                                                       in_=b2_d[:]))

        def drain_filler(k=1):
            for _ in range(k):
                if fillers:
                    fillers.pop(0)()

        # ================= layer 0: propagate-then-project =================
        for b in range(NBLK):
            l0_project_block(0, b)
        for p, (tin, tout) in enumerate(
                [("l0h0", "l0h1"), ("l0h1", "l0h2")], start=1):
            for b in range(NBLK):
                nb = _nb_of(b)
                (_, _, ps), = seg_psums(tin, b)
                stg = work.tile([128, H], f16, tag="stage")
                nc.vector.tensor_scalar_mul(stg[:, :H], ps[:, :H],
                                            dinv_sb[:, b, 0:1])
                nc.vector.tensor_scalar_mul(scall[:nb, b, :], ps[:nb, :H],
                                            dinv_sb[:nb, b, 1:2])
                stage_ag(tout, b, scall[:nb, b, :], nb, agin=False)
                transpose_pack(2 * p, [stg[:nb, kt * 128:(kt + 1) * 128]
                                       for kt in range(2)], b, nb)
                l0_project_block(p, b)
            flush_agin_big(tout)
            allgather(tout)

        def proj_block(li, b):
            """Layer li+1 staged power projections (p=1..3) for block b from
            hT, scaled into the z1 staging row, then stage z1. The p=0
            projection is deferred into the (DMA-bound) z1-hop phase — its
            output is only read by the next layernorm. Powers are packed two
            per PSUM bank so each block uses 2 "mm" ring slots, leaving WAR
            slack for the next block's segment matmuls."""
            nb = _nb_of(b)
            zname = f"l{li + 1}z1"
            zdt1 = tspec[zname][1]
            w12_sb = w12_sbs[li]
            ztile = zpool.tile([128, PH], zdt1, tag="zstage")
            for pair in range(2):
                ps = psum.tile([128, 512], f32, tag="mm")
                for half in range(2):
                    p = 2 * pair + half
                    if p == 0:
                        continue
                    for kt in range(8):
                        nc.tensor.matmul(
                            ps[:nb, half * H:(half + 1) * H],
                            hT[:, kt, b * BLK: b * BLK + nb],
                            w12_sb[:, p, kt, :],
                            start=(kt == 0), stop=(kt == 7))
                if pair == 0:
                    nc.vector.tensor_scalar_mul(
                        ztile[:nb, 0:H], ps[:nb, H: 2 * H],
                        dinv_sb[:nb, b, 2:3])
                else:
                    nc.vector.tensor_scalar_mul(
                        ztile[:nb, H: 3 * H], ps[:nb, : 2 * H],
                        dinv_sb[:nb, b, 2:3])
            stage_ag(zname, b, ztile[:nb, : 3 * H], nb)

        def proj_p0(li, b):
            """Deferred p=0 projection into hcat[:, b, 0:H]."""
            nb = _nb_of(b)
            w12_sb = w12_sbs[li]
            ps = psum.tile([128, 512], f32, tag="mm")
            for kt in range(8):
                nc.tensor.matmul(ps[:nb, :H],
                                 hT[:, kt, b * BLK: b * BLK + nb],
                                 w12_sb[:, 0, kt, :],
                                 start=(kt == 0), stop=(kt == 7))
            nc.vector.tensor_copy(hcat[:nb, b, 0:H], ps[:nb, :H])

        # ======= fused transition: layer-i last hop -> layer-(i+1) projs ====
        # Per block: segment hop, hop output into hcat, ln stats + gelu
        # (DVE/ACT); one block later (so PE never waits on the gelu chain):
        # transposes into hT and the next layer's projections + z1 staging.
        def transition(layer):
            """layer 0: hop over l0h2 (power 3) + project into l1z1.
            layer 1: hop over l1z3 + project into l2z1."""
            def stage_a(b):
                nb = _nb_of(b)
                ln_stats_half0(layer, b)
                if layer == 0:
                    (_, _, ps), = seg_psums("l0h2", b)
                    drain_filler(4 if b < 2 else 2)
                    stg = work.tile([128, H], f16, tag="stage")
                    nc.vector.tensor_scalar_mul(stg[:, :H], ps[:, :H],
                                                dinv_sb[:, b, 0:1])
                    transpose_pack(6, [stg[:nb, kt * 128:(kt + 1) * 128]
                                       for kt in range(2)], b, nb)
                    l0_project_block(3, b)
                else:
                    (_, _, ps), = seg_psums("l1z3", b)
                    drain_filler(4 if b < 2 else 2)
                    nc.vector.tensor_scalar_mul(
                        hcat[:nb, b, 3 * H: 4 * H], ps[:nb, :H],
                        dinv_sb[:nb, b, 0:1])
                ln_stats(layer, b)
                ln_gelu(layer, b)

            def stage_b(b):
                ln_transposes(b)
                proj_block(layer, b)

            for b in range(NBLK):
                stage_a(b)
                if b >= 2:
                    stage_b(b - 2)
            stage_b(NBLK - 2)
            stage_b(NBLK - 1)
            allgather(f"l{layer + 1}z1")

        transition(0)

        # ================= layers 1-2: wide hops ===========================
        def wide_hops(layer):
            """Hops 768 -> 512 over the scaled fp8 z tables. PSUM carries
            s_in * A z_in; copies out rescale: hcat gets 1/s_in, staging
            gets s_out/s_in."""
            zname = [f"l{layer}z1", f"l{layer}z2", f"l{layer}z3"]
            for hop in range(2):
                width = (3 - hop) * H
                tin, tout = zname[hop], zname[hop + 1]
                for b in range(NBLK):
                    nb = _nb_of(b)
                    pieces = seg_psums(tin, b)
                    if hop == 0:
                        proj_p0(layer - 1, b)
                    hc_col = {0: 3, 1: 5}[hop]
                    nc.vector.tensor_scalar_mul(
                        hcat[:nb, b, (hop + 1) * H:(hop + 2) * H],
                        pieces[0][2][:nb, :H],
                        dinv_sb[:nb, b, hc_col: hc_col + 1])
                    zdt_o = tspec[tout][1]
                    stg = zpool.tile([128, 2 * H], zdt_o, tag="zhstage")
                    st_col = 4 if hop == 0 else 6
                    for (c0, cw, ps) in pieces:
                        if c0 + cw <= H:
                            continue
                        lo = max(H, c0)
                        nc.vector.tensor_scalar_mul(
                            stg[:nb, lo - H: c0 + cw - H],
                            ps[:nb, lo - c0: cw],
                            dinv_sb[:nb, b, st_col: st_col + 1])
                    stage_ag(tout, b, stg[:nb, : width - H], nb)
                allgather(tout)

        wide_hops(1)
        transition(1)
        wide_hops(2)

        # ==== final: layer-2 last hop fused with layernorm + MLP chunks ====
        m1T = big.tile([128, 2, NB], f16, tag="m1T")
        m2T = big.tile([128, NB], f16, tag="m2T")
        ysb = big.tile([1, NB], f32, tag="ysb")

        def mlp_chunk(c0, cw):
            for mt in range(2):
                ps = psum.tile([128, 512], f32, tag="mm")
                for kt in range(8):
                    nc.tensor.matmul(ps[:, :cw], w1_sb[:, kt, mt, :],
                                     hT[:, kt, c0:c0 + cw],
                                     start=(kt == 0), stop=(kt == 7))
                bias = b1_sb[:, mt:mt + 1] if nontriv["b1"] else zb(128)
                nc.scalar.activation(m1T[:, mt, c0:c0 + cw], ps[:, :cw],
                                     AF.Gelu, bias=bias)
            ps = psum.tile([128, 512], f32, tag="mm")
            for kt in range(2):
                nc.tensor.matmul(ps[:, :cw], w2_sb[:, kt, 0, :],
                                 m1T[:, kt, c0:c0 + cw],
                                 start=(kt == 0), stop=(kt == 1))
            bias = b2_sb[:, 0:1] if nontriv["b2"] else zb(128)
            nc.scalar.activation(m2T[:, c0:c0 + cw], ps[:, :cw],
                                 AF.Gelu, bias=bias)
            ps = psum.tile([128, 512], f32, tag="mm")
            nc.tensor.matmul(ps[:1, :cw], w3_sb[:, :1], m2T[:, c0:c0 + cw],
                             start=True, stop=True)
            nc.vector.tensor_copy(ysb[:1, c0:c0 + cw], ps[:1, :cw])

        def final_a(b):
            nb = _nb_of(b)
            ln_stats_half0(2, b)
            (_, _, ps), = seg_psums("l2z3", b)
            drain_filler(2)
            nc.vector.tensor_scalar_mul(
                hcat[:nb, b, 3 * H: 4 * H], ps[:nb, :H],
                dinv_sb[:nb, b, 0:1])
            ln_stats(2, b)
            ln_gelu(2, b)

        mlp_chunks = [(0, 512), (512, 512), (1024, 128), (1152, 98)]

        def final_b(b):
            ln_transposes(b)
            ready = min((b + 1) * BLK, NB)
            while mlp_chunks and mlp_chunks[0][0] + mlp_chunks[0][1] <= ready:
                c0, cw = mlp_chunks.pop(0)
                mlp_chunk(c0, cw)

        for b in range(NBLK):
            final_a(b)
            if b >= 1:
                final_b(b - 1)
        final_b(NBLK - 1)
        nc.sync.dma_start(out=y_d[:], in_=ysb[:1, :])

    nc.compile()
    return nc


# ----------------------------------------------------------------------------
# Public entry point
# ----------------------------------------------------------------------------

_CACHE = {}


def _prep_inputs(inputs):
    x = np.asarray(inputs["x"], np.float32)
    edge_index = np.asarray(inputs["edge_index"])
    wcnt, dvec, idx16, k_pad, tbmax, perm = _graph_prep(edge_index)

    b3 = np.asarray(inputs["b3"], np.float32)
    nontriv = {
        "b_in": bool(np.any(inputs["b_in"])),
        "bcat": bool(np.any(inputs["mh_b0"]) or np.any(inputs["mh_b12"])),
        "ln": not (np.allclose(np.asarray(inputs["ln_g"]), 1.0)
                   and not np.any(inputs["ln_b"])),
        "b1": bool(np.any(inputs["b1"])),
        "b2": bool(np.any(inputs["b2"])),
    }

    shared = {
        "w_in_st": _w_stationary(np.asarray(inputs["w_in"], np.float32)),
        "w0_m": np.stack([_w_moving(np.asarray(inputs["mh_w0"][p], np.float32))
                          for p in range(P4)]),
        "w12_m": np.stack([
            np.stack([_w_moving(np.asarray(inputs["mh_w12"][l, p], np.float32))
                      for p in range(P4)])
            for l in range(2)]),
        "w1_st": _w_stationary(np.asarray(inputs["w1"], np.float32)),
        "w2_st": _w_stationary(np.asarray(inputs["w2"], np.float32)),
        "w3_st": np.asarray(inputs["w3"], np.float32).astype(np.float16),
        "ident": np.eye(128, dtype=np.float16),
    }
    if nontriv["b_in"]:
        shared["b_in_fm"] = np.ascontiguousarray(
            np.asarray(inputs["b_in"], np.float32).reshape(2, 128).T)
        shared["b_in_bc"] = np.tile(np.asarray(inputs["b_in"], np.float32),
                                    (128, 1))
    if nontriv["bcat"]:
        bcat = np.zeros((L, PH), np.float32)
        bcat[0] = np.asarray(inputs["mh_b0"], np.float32).reshape(-1)
        bcat[1] = np.asarray(inputs["mh_b12"], np.float32)[0].reshape(-1)
        bcat[2] = np.asarray(inputs["mh_b12"], np.float32)[1].reshape(-1)
        shared["bcat_bc"] = np.ascontiguousarray(
            np.broadcast_to(bcat[:, None, :], (L, 128, PH)))
    if nontriv["ln"]:
        shared["lng_bc"] = np.ascontiguousarray(np.broadcast_to(
            np.asarray(inputs["ln_g"], np.float32)[:, None, :], (L, 128, PH)))
        shared["lnb_bc"] = np.ascontiguousarray(np.broadcast_to(
            np.asarray(inputs["ln_b"], np.float32)[:, None, :], (L, 128, PH)))
    if nontriv["b1"]:
        shared["b1_c"] = np.ascontiguousarray(
            np.asarray(inputs["b1"], np.float32).reshape(2, 128).T)
    if nontriv["b2"]:
        shared["b2_c"] = np.asarray(inputs["b2"], np.float32).reshape(128, 1)

    xp = x[np.argsort(perm)]  # xp[newid] = x[orig]
    in_maps = []
    for c in range(NC):
        m = dict(shared)
        m["xT"] = np.ascontiguousarray(
            xp[c * NB:(c + 1) * NB].T.astype(np.float16))
        m["idx16"] = np.ascontiguousarray(idx16[c])
        m["wsegT"] = np.ascontiguousarray(wcnt[c])
        m["dinv_c"] = np.ascontiguousarray(dvec[c])
        in_maps.append(m)
    return in_maps, k_pad, tbmax, nontriv, b3, perm


def _run(inputs, trace=False, **kwargs):
    from concourse.bass_utils import run_bass_kernel_spmd

    in_maps, k_pad, tbmax, nontriv, b3, perm = _prep_inputs(inputs)
    key = (k_pad, tbmax, tuple(sorted(nontriv.items())))
    if key not in _CACHE:
        _CACHE[key] = _build_nc(k_pad, tbmax, nontriv)
    nc = _CACHE[key]
    res = run_bass_kernel_spmd(nc, in_maps, list(range(NC)), trace=trace,
                               **kwargs)
    ycat = np.concatenate([res.results[c]["y_out"] for c in range(NC)])
    y = ycat[perm].astype(np.float32) + b3.reshape(-1)[0]
    return y, res


def kernel(**inputs) -> np.ndarray:
    y, _ = _run(inputs, trace=False)
    return y
